# revision 59
# baseline (speedup 1.0000x reference)
"""Trainium2 Bass kernel for nn_MultiHeadAttention_44281112822190.

8 NeuronCores, pure data parallelism over the 8192 (b,s) rows: core c takes
rows [c*1024, (c+1)*1024) (batch b = c//2, s-offset (c%2)*1024). No
collectives; the host shards inputs and reassembles the output.

Math notes:
  - The reference applies RoPE to q and k, then contracts q.k at the SAME
    position (per-position head-head attention [B,S,H,H]). RoPE is an
    orthogonal per-position rotation applied identically to q and k, so it
    cancels exactly in the scores: (R q).(R k) = q.k. The kernel skips RoPE
    entirely (freqs inputs are unused).
  - The reference's "h-major flatten" transpose(0,2,1,3).reshape(B,S,-1) is a
    scramble: out[b, h*128 + s//16, (s%16)*128 + d] = att_out[b, s, h, d].
    Each scrambled row draws from 16 consecutive positions of one head, all
    inside one core's shard, so the output projection stays core-local.

Numerics: all matmul operands are fp16 with fp32 PSUM accumulation ->
~7e-4 relative error end-to-end, 1 cycle/row on the PE.

Per-core schedule (one 1024-position block) — keeps the PE saturated
end-to-end (the original kernel had a serial attention phase that idled
the PE and dropped the HAM clock gate to 1.2 GHz):
  1. Startup: 48 dummy warm-up matmuls on a memset tile keep the HAM clock
     gate open while x streams in as 16 x 256KB DMAs across the 8 queues;
     the first Q weight tile is issued ahead, split into two halves so the
     k<8 matmuls start on the first half (first real matmul ~13us vs ~29us).
  2. Q then K projections (stationary = host-transposed weight tiles,
     moving = host-transposed x). Q is [128 d, 1024 s, 16 h]; K and V use
     a packed [128 d, 128 s//8, 128 (h*8+s%8)] layout whose copybacks
     write 8-element-contiguous runs (~4x cheaper on DVE than stride-16
     h-minor writes) while attention slabs stay single-free-dim.
  3. V projection in TWO position-half passes. V-lo (positions 0:512)
     interleaved with attention stage1 of quarter 0 (scores matmul + exp +
     fused mask/den + reciprocal + normalize -> att2 fp16 in a 20-deep
     SBUF ring; needs only Q,K). V-hi interleaved with stage2 of quarter 0
     (att/V-slab PE transposes + attO matmul + scatter into attO tiles;
     needs only V-lo positions) and stage1 of quarter 1.
  4. Quarter-shifted pipeline: finals(q) || stage2(q+1) || stage1(q+2).
     stage2 is itself split: the attO matmul runs one pair behind the
     transposes so the ScalarE trsb copy has a full cycle of slack.
     Within each output chunk, stage2/stage1 are emitted before the final
     matmuls. Engine split: ScalarE does exp, transpose copyback, the
     final bias-add and one V-hi copyback (emitted after the stage ops);
     Vector does mask+den, scatter and the other bias copybacks; GpSimd
     does the softmax normalize.
  5. DMA: f3 would need ~368 GB/s (512KB wot + 128KB out per 1.74us
     chunk) against a ~358 GB/s core budget, so after V-hi the dead x
     SBUF (32KB/partition) is reused via same-tag tile reallocation to
     hold wot tiles t2=4..7 resident (loaded once, used by all four
     quarters: 20MB of wot traffic instead of 32MB). Streamed wot tiles
     are fetched as two half-DMAs on two queues (one queue's ~118 GB/s
     cannot keep up).
  PSUM (8 banks): ppj 4x[128,512] ring shared by projections and finals;
  static ping-pong tiles for scores (2x256), attO (2x256), transposes
  (4x512 fp16). ob staging ring of 4 absorbs out-DMA queueing behind
  weight-tile DMAs.
Host reassembles the scrambled rows into the final [4, 2048, 2048] output.

Measured on trn2 (8 cores): ~507us median / 506us min HW exec
(run-to-run device power state varies +-5%; always bench several runs),
rel err 6.7e-4. Baseline from the previous session: 584us.
"""

import os
import sys

sys.path.insert(0, "/opt/trn_rl_repo")

import numpy as np

import concourse.bacc as bacc
import concourse.mybir as mybir
import concourse.tile as tile
from concourse.bass_utils import run_bass_kernel_spmd

F32 = mybir.dt.float32
F16 = mybir.dt.float16
AF = mybir.ActivationFunctionType
ALU = mybir.AluOpType

B, S, E, H, D = 4, 2048, 2048, 16, 128
NCORES = 8
SCALE = 1.0 / float(np.sqrt(D))

_CACHE = {}
LAST_EXEC_NS = None


def _build():
    nc = bacc.Bacc(trn_type="TRN2", target_bir_lowering=False)

    xt = nc.dram_tensor("xt", [16, 128, 1024], F16, kind="ExternalInput")
    wqt = nc.dram_tensor("wqt", [E, E], F16, kind="ExternalInput")
    wkt = nc.dram_tensor("wkt", [E, E], F16, kind="ExternalInput")
    wvt = nc.dram_tensor("wvt", [E, E], F16, kind="ExternalInput")
    wot = nc.dram_tensor("wot", [E, E], F16, kind="ExternalInput")
    bqt = nc.dram_tensor("bqt", [128, 16], F32, kind="ExternalInput")
    bkt = nc.dram_tensor("bkt", [128, 16], F32, kind="ExternalInput")
    bvt = nc.dram_tensor("bvt", [128, 16], F32, kind="ExternalInput")
    bot = nc.dram_tensor("bot", [128, 16], F32, kind="ExternalInput")
    mask01 = nc.dram_tensor("mask01", [128, 128], F32, kind="ExternalInput")
    ident = nc.dram_tensor("ident", [128, 128], F16, kind="ExternalInput")
    out = nc.dram_tensor("out", [16, 128, 1024], F32, kind="ExternalOutput")

    with tile.TileContext(nc) as tc:
        with (
            tc.tile_pool(name="const", bufs=1) as cp,
            tc.tile_pool(name="xp", bufs=1) as xp,
            tc.tile_pool(name="qkv", bufs=1) as qkvp,
            tc.tile_pool(name="aop", bufs=1) as aop,
            tc.tile_pool(name="wp", bufs=3) as wp,
            tc.tile_pool(name="gp", bufs=3) as gp,
            tc.tile_pool(name="a2p", bufs=20) as a2p,
            tc.tile_pool(name="op", bufs=4) as op,
            tc.tile_pool(name="ppj", bufs=4, space="PSUM") as ppj,
            tc.tile_pool(name="psm", bufs=1, space="PSUM") as psm,
        ):
            # static PSUM ping-pong rings (subtile deps order the reuse):
            # scores (2-deep), attO (2-deep), transposes (4-deep, 2 banks);
            # finals share the 4-deep ppj pool with the projections
            ga_t = psm.tile([128, 2, 256], F32, tag="ga", name="ga_t")
            po_t = psm.tile([128, 2, 256], F32, tag="po", name="po_t")
            tr_t = psm.tile([128, 4, 512], F16, tag="tr", name="tr_t")

            # warm-up: keep the PE busy (and the HAM clock gate open) while
            # the x DMA streams in; results are discarded
            warm = cp.tile([128, 128], F16, tag="warm")
            nc.gpsimd.memset(warm[:], 0.0)
            warm_ps = ppj.tile([128, 512], F32, tag="pp", name="warm_ps")
            for _ in range(48):
                nc.tensor.matmul(
                    warm_ps[:, 0:128], warm[:], warm[:], start=True, stop=True
                )
            # first Q weight tile ahead of everything (split in two so the
            # k<8 matmuls can start on the first half): it shares the DMA
            # critical path with x
            w0 = wp.tile([128, 16, 256], F16, tag="w", name="w0")
            xtb_c = []
            # x pieces 0-1 first (the first matmul needs x0 + w0-lo; x0 is
            # the smaller transfer so let it win a queue), then the w0
            # halves, then the rest of x
            for kc in range(2):
                xc = xp.tile([128, 1024], F16, tag=f"xtb{kc}", name=f"xtb{kc}")
                nc.sync.dma_start(xc[:], xt[kc, :, :])
                xtb_c.append(xc)
            nc.sync.dma_start(
                w0[:, 0:8, :],
                wqt[0:1024, 0:256].rearrange("(k p) c -> p k c", p=128),
            )
            nc.sync.dma_start(
                w0[:, 8:16, :],
                wqt[1024:2048, 0:256].rearrange("(k p) c -> p k c", p=128),
            )
            for kc in range(2, 16):
                xc = xp.tile([128, 1024], F16, tag=f"xtb{kc}", name=f"xtb{kc}")
                nc.sync.dma_start(xc[:], xt[kc, :, :])
                xtb_c.append(xc)

            mask_sb = cp.tile([128, 128], F32, tag="mask")
            id_sb = cp.tile([128, 128], F16, tag="id")
            nc.sync.dma_start(mask_sb[:], mask01[:, :])
            nc.sync.dma_start(id_sb[:], ident[:, :])
            bias_sb = {}
            for name, t_ in (("bq", bqt), ("bk", bkt), ("bv", bvt), ("bo", bot)):
                b_sb = cp.tile([128, 16], F32, tag=name, name=name)
                nc.sync.dma_start(b_sb[:], t_[:, :])
                bias_sb[name] = b_sb

            # --- projections: qb is [128 d, 1024 s, 16 h] (h-minor); kb/vb
            # use a packed layout [128 d, 128 s//8, 128 (h*8 + s%8)] whose
            # copyback writes are 8-element-contiguous runs (4x cheaper on
            # DVE than the h-minor stride-16 writes) while attention slabs
            # stay single-free-dim (kb[:, u2, :]), which the PE requires ---
            qb = qkvp.tile([128, 1024, 16], F16, tag="qb")
            kb = qkvp.tile([128, 128, 128], F16, tag="kb")
            vb = qkvp.tile([128, 128, 128], F16, tag="vb")

            def proj_chunk(wdram, bias, dst, t2, wtile=None, packed=False):
                if wtile is None:
                    wtile = wp.tile([128, 16, 256], F16, tag="w", name="w")
                    nc.sync.dma_start(
                        wtile[:],
                        wdram[:, t2 * 256 : (t2 + 1) * 256].rearrange(
                            "(k p) c -> p k c", p=128
                        ),
                    )
                for half in range(2):
                    t = 2 * t2 + half
                    psA = ppj.tile([128, 512], F32, tag="pp", name="psA")
                    psB = ppj.tile([128, 512], F32, tag="pp", name="psB")
                    for k in range(16):
                        w_ap = wtile[:, k, half * 128 : half * 128 + 128]
                        nc.tensor.matmul(
                            psA[:], w_ap, xtb_c[k][:, 0:512],
                            start=(k == 0), stop=(k == 15),
                        )
                        nc.tensor.matmul(
                            psB[:], w_ap, xtb_c[k][:, 512:1024],
                            start=(k == 0), stop=(k == 15),
                        )
                    for lo, ps in ((0, psA), (512, psB)):
                        if packed:
                            d_ap = dst[
                                :, lo // 8 : lo // 8 + 64, t * 8 : t * 8 + 8
                            ]
                        else:
                            d_ap = dst[:, lo : lo + 512, t]
                        if lo and t2 < 2 and dst is qb:
                            # startup only: Scalar runs the psB copyback in
                            # parallel with DVE's psA so the ppj ring frees
                            # ~2us sooner while the pipeline fills
                            nc.scalar.activation(
                                d_ap, ps[:], AF.Identity,
                                bias=bias_sb[bias][:, t : t + 1],
                            )
                        else:
                            nc.vector.tensor_scalar_add(
                                d_ap, ps[:], bias_sb[bias][:, t : t + 1]
                            )

            def proj_half(wdram, bias, dst, t2, ph):
                # one position-half (512 cols) of both heads 2*t2, 2*t2+1;
                # used for V so attention stage2 can start after the lo pass
                wtile = wp.tile([128, 16, 256], F16, tag="w", name="w")
                nc.sync.dma_start(
                    wtile[:],
                    wdram[:, t2 * 256 : (t2 + 1) * 256].rearrange(
                        "(k p) c -> p k c", p=128
                    ),
                )
                lo = 512 * ph
                deferred = []
                for half in range(2):
                    t = 2 * t2 + half
                    ps = ppj.tile([128, 512], F32, tag="pp", name="psV")
                    for k in range(16):
                        nc.tensor.matmul(
                            ps[:],
                            wtile[:, k, half * 128 : half * 128 + 128],
                            xtb_c[k][:, lo : lo + 512],
                            start=(k == 0), stop=(k == 15),
                        )
                    d_ap = dst[:, lo // 8 : lo // 8 + 64, t * 8 : t * 8 + 8]
                    if ph == 1:
                        deferred.append((d_ap, ps, t))
                    else:
                        nc.vector.tensor_scalar_add(
                            d_ap, ps[:], bias_sb[bias][:, t : t + 1]
                        )
                return deferred

            # --- attention stages ---
            # attO half-tiles: [128 d, 16 sl, 256] with col = u_local*16 + h
            attO_h = [
                aop.tile([128, 16, 256], F16, tag=f"attO{q}", name=f"attO{q}")
                for q in range(4)
            ]
            att2_tiles = [None] * 64

            def stage1(P2):
                # scores + softmax for positions [P2*16, P2*16+16); needs qb,kb
                G = 2 * P2
                ga = ga_t[:, P2 % 2, :]
                for j in range(2):
                    s0 = (G + j) * 8
                    nc.tensor.matmul(
                        ga[:, 128 * j : 128 * j + 128],
                        qb[:, s0 : s0 + 8, :],
                        kb[:, G + j, :],
                        start=True, stop=True,
                    )
                e2 = gp.tile([128, 256], F32, tag="e2", name="e2")
                nc.scalar.activation(e2[:], ga, AF.Exp, scale=SCALE)
                em2 = e2[:].rearrange("p (g c) -> p g c", g=2)
                den2 = gp.tile([128, 2], F32, tag="den2", name="den2")
                for j in range(2):
                    nc.vector.scalar_tensor_tensor(
                        em2[:, j, :], e2[:, 128 * j : 128 * j + 128], 1.0,
                        mask_sb[:], ALU.bypass, ALU.mult,
                        accum_out=den2[:, j : j + 1],
                    )
                rec2 = gp.tile([128, 2], F32, tag="rec2", name="rec2")
                nc.vector.reciprocal(rec2[:], den2[:])
                att2 = a2p.tile([128, 2, 128], F16, tag="att2", name="att2")
                nc.gpsimd.tensor_tensor(
                    att2[:], em2, rec2[:].unsqueeze(2).to_broadcast([128, 2, 128]),
                    ALU.mult,
                )
                att2_tiles[P2] = att2

            trsb_tiles = {}

            def stage2a(P2):
                # att/V-slab transposes for positions [P2*16, P2*16+16)
                G = 2 * P2
                att2 = att2_tiles[P2]
                tr = tr_t[:, P2 % 4, :]
                for j in range(2):
                    s0 = (G + j) * 8
                    nc.tensor.transpose(
                        tr[:, 128 * j : 128 * j + 128], att2[:, j, :], id_sb[:]
                    )
                    nc.tensor.transpose(
                        tr[:, 256 + 128 * j : 384 + 128 * j],
                        vb[:, G + j, :], id_sb[:],
                    )
                trsb = gp.tile([128, 512], F16, tag="trsb", name="trsb")
                nc.scalar.activation(trsb[:], tr, AF.Copy)
                trsb_tiles[P2] = trsb

            def stage2b(P2):
                # attO matmul + scatter; runs one slot behind stage2a so the
                # ScalarE trsb copy has a full cycle of slack
                trsb = trsb_tiles.pop(P2)
                po = po_t[:, P2 % 2, :]
                for j in range(2):
                    nc.tensor.matmul(
                        po[:, 128 * j : 128 * j + 128],
                        trsb[:, 256 + 128 * j : 384 + 128 * j],
                        trsb[:, 128 * j : 128 * j + 128],
                        start=True, stop=True,
                    )
                # scatter: psum cols (g2, i, h) -> attO_h[u_hi][:, (g2,i), u_lo*16+h]
                u_hi, u_lo = P2 // 16, P2 % 16
                dst = attO_h[u_hi][:].rearrange(
                    "p (g2 i) (u h) -> p g2 i u h", g2=2, h=16
                )[:, :, :, u_lo, :]
                nc.vector.tensor_copy(dst, po)

            pending_s2 = []

            def stage2(P2):
                stage2a(P2)
                if pending_s2:
                    stage2b(pending_s2.pop())
                pending_s2.append(P2)

            def stage2_flush():
                while pending_s2:
                    stage2b(pending_s2.pop())

            def final_w_dma(t2):
                # two half-DMAs land on two queues: one queue's ~118 GB/s
                # cannot keep up with the f-phase consumption rate
                wtile = wp.tile([128, 16, 256], F16, tag="w", name="w")
                for h2 in range(2):
                    nc.sync.dma_start(
                        wtile[:, 8 * h2 : 8 * h2 + 8, :],
                        wot[
                            1024 * h2 : 1024 * h2 + 1024,
                            t2 * 256 : (t2 + 1) * 256,
                        ].rearrange("(k p) c -> p k c", p=128),
                    )
                return wtile

            # after V-hi the x tiles are dead; reuse their 32KB/partition of
            # SBUF (same-tag reallocation) to hold wot tiles t2=4..7
            # resident, loaded once instead of re-streamed every quarter --
            # f3 otherwise needs 368 GB/s of DMA (> the 358 core budget)
            wres = {}

            def load_resident(t2s):
                for t2 in t2s:
                    pieces = []
                    for j in range(4):
                        tag = 4 * (t2 - 4) + j
                        pc = xp.tile(
                            [128, 4, 256], F16, tag=f"xtb{tag}",
                            name=f"wres{t2}_{j}",
                        )
                        nc.sync.dma_start(
                            pc[:],
                            wot[
                                512 * j : 512 * j + 512,
                                t2 * 256 : (t2 + 1) * 256,
                            ].rearrange("(k p) c -> p k c", p=128),
                        )
                        pieces.append(pc)
                    wres[t2] = pieces

            def final_t(q, t2, half, wtile=None, pieces=None):
                t = 2 * t2 + half
                psf = ppj.tile([128, 512], F32, tag="pp", name="psf")
                ps = psf[:, 0:256]
                for sl in range(16):
                    if wtile is not None:
                        w_ap = wtile[:, sl, half * 128 : half * 128 + 128]
                    else:
                        w_ap = pieces[sl // 4][
                            :, sl % 4, half * 128 : half * 128 + 128
                        ]
                    nc.tensor.matmul(
                        ps, w_ap, attO_h[q][:, sl, :],
                        start=(sl == 0), stop=(sl == 15),
                    )
                ob = op.tile([128, 256], F32, tag="ob", name="ob")
                nc.scalar.activation(
                    ob[:], ps, AF.Identity, bias=bias_sb["bo"][:, t : t + 1]
                )
                nc.sync.dma_start(out[t, :, q * 256 : q * 256 + 256], ob[:])

            # --- main schedule ---
            # quarter-shifted pipeline: finals(q) || stage2(q+1) || stage1(q+2)
            proj_chunk(wqt, "bq", qb, 0, wtile=w0)
            for t2 in range(1, 8):
                proj_chunk(wqt, "bq", qb, t2)
            for t2 in range(8):
                proj_chunk(wkt, "bk", kb, t2, packed=True)
            for t2 in range(8):  # V positions 0:512 || stage1(q0)
                proj_half(wvt, "bv", vb, t2, 0)
                stage1(2 * t2)
                stage1(2 * t2 + 1)
            for t2 in range(8):  # V positions 512:1024 || stage2(q0), stage1(q1)
                deferred = proj_half(wvt, "bv", vb, t2, 1)
                stage2(2 * t2)
                stage2(2 * t2 + 1)
                stage1(16 + 2 * t2)
                stage1(16 + 2 * t2 + 1)
                # copybacks after the stage ops, both on DVE (whose stt work
                # moved to GpSimd): a ScalarE copyback here delays the trsb
                # copies that free the tr PSUM ring and stalls the PE
                for d_ap, psd, td in deferred:
                    nc.vector.tensor_scalar_add(
                        d_ap, psd[:], bias_sb["bv"][:, td : td + 1]
                    )
            stage2_flush()
            for q in range(4):
                res_from = 6 if q == 0 else 4
                for i in range(8):
                    if i < res_from:
                        wtile, pieces = final_w_dma(i), None
                    else:
                        wtile, pieces = None, wres[i]
                    # resident loads go out AFTER the phase's first streamed
                    # tile so that tile wins the DMA-queue race (observed
                    # ~2.7us LDW stalls at f0/f1 starts otherwise)
                    if i == 0 and q == 0:
                        load_resident([6, 7])
                    if i == 0 and q == 1:
                        load_resident([4, 5])
                    for half in range(2):
                        idx = 16 * q + 2 * i + half
                        if q < 3:
                            stage2(idx + 16)
                        if q < 2:
                            stage1(idx + 32)
                        final_t(q, i, half, wtile, pieces)
                stage2_flush()

    nc.compile()
    return nc


def _get_nc():
    if "nc" not in _CACHE:
        _CACHE["nc"] = _build()
    return _CACHE["nc"]


def make_in_maps(inputs):
    x = np.ascontiguousarray(np.asarray(inputs["x"], dtype=np.float32))
    ws = {k: np.asarray(inputs[k], dtype=np.float32) for k in ("wq", "wk", "wv", "wo")}
    bs = {k: np.asarray(inputs[k], dtype=np.float32) for k in ("bq", "bk", "bv", "bo")}

    xf = x.reshape(B * S, E)
    f16 = lambda a: np.ascontiguousarray(a).astype(np.float16)
    btile = lambda b: np.ascontiguousarray(b.reshape(16, 128).T)
    # score rows are (i,h) i-major (from h-minor qb slabs); score cols are
    # (g,i) from the packed kb slabs, so col position index = c % 8
    ii = np.arange(128) // 16
    jj = np.arange(128) % 8
    mask01 = (ii[:, None] == jj[None, :]).astype(np.float32)
    common = {
        "wqt": f16(ws["wq"].T), "wkt": f16(ws["wk"].T),
        "wvt": f16(ws["wv"].T), "wot": f16(ws["wo"].T),
        "bqt": btile(bs["bq"]), "bkt": btile(bs["bk"]),
        "bvt": btile(bs["bv"]), "bot": btile(bs["bo"]),
        "mask01": mask01, "ident": np.eye(128, dtype=np.float16),
    }
    in_maps = []
    for c in range(NCORES):
        xt_c = f16(xf[c * 1024 : (c + 1) * 1024].T).reshape(16, 128, 1024)
        in_maps.append({"xt": xt_c, **common})
    return in_maps


def assemble(results):
    out = np.empty((B, S, E), np.float32)
    for c in range(NCORES):
        O = results[c]["out"]  # [16 t, 128 p, 1024]; col = u*16 + h
        Oc = O.reshape(E, 64, 16)  # [j, u, h]
        tgt = out[c // 2].reshape(16, 128, E)
        v0 = (c % 2) * 64
        tgt[:, v0 : v0 + 64, :] = Oc.transpose(2, 1, 0)
    return out


def kernel(**inputs):
    global LAST_EXEC_NS
    nc = _get_nc()
    res = run_bass_kernel_spmd(nc, make_in_maps(inputs), core_ids=list(range(NCORES)))
    LAST_EXEC_NS = res.exec_time_ns
    return assemble(res.results)
